# revision 3
# baseline (speedup 1.0000x reference)
"""Bilinear resampling kernel for Trainium2 (8 NeuronCores, SPMD).

reference semantics:
    u = target_uv[:, 0] / downscale ; v = target_uv[:, 1] / downscale
    out[c, i] = bilinear sample of feature_map[c] at (v[i], u[i])   -> [C, N]

Strategy
--------
Host: sort points by v, split into 8 equal per-core chunks, then split each
core's chunk into B equal "bands" (each spanning <= ~24 feature-map rows so
that pixel indices within a band fit in int16 for dma_gather).  Each band's
map rows are sliced out (width padded 1248->1280) and shipped per core.

Device (same program on all 8 cores):
  phase 1: transpose the band map slices [C=128, SPAN*1280] -> [SPAN*1280, C]
           (pixel-major, 512B per pixel) into DRAM scratch via PE transposes.
  phase 2: per chunk of points, one dma_gather pulls a 1KB window per
           (point, v-row): the two u-corner pixel vectors, contiguous.
  phase 3: blend with per-partition scalar weights (4 fused DVE ops/tile),
           PE-transpose each [pts, C] tile back to [C, pts], copy via ScalarE
           into an output buffer, DMA to out.

Layouts (per band, Tb tiles of 128 points):
  gather i = j*256 + c*128 + p   (j tile, c in {v_lo, v_hi}, p partition)
  dma_gather output[p, i//128, :] = window  ->  G[p, 2j+c, 0:256]
  idx int16 at [i%16, i//16]  ->  col 16j + 8c + a  for p = 16a + p2;
  host pre-wraps uv accordingly (replicated over the 8 partition groups so
  the computed idx tile is replicated as the HW requires).
"""

import numpy as np

import concourse.bacc as bacc
import concourse.bass as bass
import concourse.mybir as mybir
import concourse.tile as tile
from concourse.bass_utils import run_bass_kernel_spmd
from concourse.masks import make_identity

C = 128
P = 128
W2 = 1280          # padded row pitch in pixels
N_CORES = 8
CT = 4             # tiles (of 128 points) per gather chunk -> 1024 idxs (SWDGE ring cap)
F32 = mybir.dt.float32
I16 = mybir.dt.int16
I32 = mybir.dt.int32


def _floor_ops(nc, pool, x_ap, n, lo_out, frac_out=None):
    """lo_out = floor(x), frac_out = x - floor(x). x >= 0. Exact regardless of
    the HW float->int rounding mode."""
    xi = pool.tile([P, n], I32, tag="flt_xi")
    xf = pool.tile([P, n], F32, tag="flt_xf")
    gt = pool.tile([P, n], F32, tag="flt_gt")
    nc.vector.tensor_copy(xi[:], x_ap)                   # f32 -> i32 (round?)
    nc.vector.tensor_copy(xf[:], xi[:])                  # i32 -> f32 exact
    nc.vector.tensor_tensor(gt[:], xf[:], x_ap, op=mybir.AluOpType.is_gt)
    nc.vector.tensor_tensor(lo_out, xf[:], gt[:], op=mybir.AluOpType.subtract)
    if frac_out is not None:
        nc.vector.tensor_tensor(frac_out, x_ap, lo_out, op=mybir.AluOpType.subtract)


def build_program(B, SPAN, Tb, num_devices=N_CORES):
    """Build the SPMD Bass program. Returns (nc, io names)."""
    Tc = B * Tb                 # total tiles per core
    NCAP = Tc * P               # point capacity per core
    JW = Tb * 8                 # wrapped (j, a) cols per band
    IDXW = Tb * 16              # idx cols per band
    NPIX = SPAN * W2            # pixels per band table

    nc = bacc.Bacc("TRN2", target_bir_lowering=False, debug=False,
                   num_devices=num_devices)

    fms = nc.dram_tensor("fms", [C, B * SPAN * W2], F32, kind="ExternalInput")
    uv_t = nc.dram_tensor("uv_t", [P, 2 * Tc], F32, kind="ExternalInput")
    uv_w = nc.dram_tensor("uv_w", [P, B * 2 * JW], F32, kind="ExternalInput")
    out = nc.dram_tensor("out", [C, NCAP], F32, kind="ExternalOutput")

    with tile.TileContext(nc) as tc:
        with (
            tc.tile_pool(name="const", bufs=1) as cpool,
            tc.tile_pool(name="wbuf", bufs=1) as wpool,
            tc.tile_pool(name="scratch", bufs=2) as spool,
            tc.tile_pool(name="rowio", bufs=3) as rpool,
            tc.tile_pool(name="gather", bufs=2) as gpool,
            tc.tile_pool(name="acc", bufs=8) as apool,
            tc.tile_pool(name="obuf", bufs=2) as opool,
            tc.tile_pool(name="psum1", bufs=3, space="PSUM") as ppool1,
            tc.tile_pool(name="psum2", bufs=4, space="PSUM") as ppool2,
            tc.tile_pool(name="dram", bufs=1, space="DRAM") as dpool,
        ):
            ident = cpool.tile([P, P], F32, tag="ident")
            make_identity(nc, ident[:])

            # ---- weights preamble: w00..w11 [P, Tc] ----
            uvt = wpool.tile([P, 2 * Tc], F32, tag="uvt")
            nc.sync.dma_start(out=uvt[:], in_=uv_t[:])
            du = wpool.tile([P, Tc], F32, tag="du")
            dv = wpool.tile([P, Tc], F32, tag="dv")
            om_du = wpool.tile([P, Tc], F32, tag="om_du")
            om_dv = wpool.tile([P, Tc], F32, tag="om_dv")
            lo_tmp = wpool.tile([P, Tc], F32, tag="lo_tmp")
            _floor_ops(nc, spool, uvt[:, :Tc], Tc, lo_tmp[:], du[:])
            _floor_ops(nc, spool, uvt[:, Tc:], Tc, lo_tmp[:], dv[:])
            nc.vector.tensor_scalar(om_du[:], du[:], -1.0, 1.0,
                                    op0=mybir.AluOpType.mult,
                                    op1=mybir.AluOpType.add)
            nc.vector.tensor_scalar(om_dv[:], dv[:], -1.0, 1.0,
                                    op0=mybir.AluOpType.mult,
                                    op1=mybir.AluOpType.add)
            w00 = wpool.tile([P, Tc], F32, tag="w00")
            w10 = wpool.tile([P, Tc], F32, tag="w10")
            w01 = wpool.tile([P, Tc], F32, tag="w01")
            w11 = wpool.tile([P, Tc], F32, tag="w11")
            nc.vector.tensor_tensor(w00[:], om_dv[:], om_du[:], op=mybir.AluOpType.mult)
            nc.vector.tensor_tensor(w10[:], dv[:], om_du[:], op=mybir.AluOpType.mult)
            nc.vector.tensor_tensor(w01[:], om_dv[:], du[:], op=mybir.AluOpType.mult)
            nc.vector.tensor_tensor(w11[:], dv[:], du[:], op=mybir.AluOpType.mult)

            # ---- per-band gather indices (int16, wrapped+replicated) ----
            idx16 = []
            for b in range(B):
                uvw = spool.tile([P, 2 * JW], F32, tag="uvw")
                nc.sync.dma_start(out=uvw[:], in_=uv_w[:, b * 2 * JW:(b + 1) * 2 * JW])
                ulo = spool.tile([P, JW], F32, tag="ulo")
                vlo = spool.tile([P, JW], F32, tag="vlo")
                _floor_ops(nc, spool, uvw[:, :JW], JW, ulo[:])
                _floor_ops(nc, spool, uvw[:, JW:], JW, vlo[:])
                pix = spool.tile([P, JW], F32, tag="pix")
                nc.vector.tensor_scalar_mul(pix[:], vlo[:], float(W2))
                nc.vector.tensor_tensor(pix[:], pix[:], ulo[:], op=mybir.AluOpType.add)
                pix_hi = spool.tile([P, JW], F32, tag="pix_hi")
                nc.vector.tensor_scalar_add(pix_hi[:], pix[:], float(W2))
                it = wpool.tile([P, IDXW], I16, tag=f"idx{b}")
                itv = it[:].rearrange("p (j c a) -> p j c a", c=2, a=8)
                pv = pix[:].rearrange("p (j a) -> p j a", a=8)
                pvh = pix_hi[:].rearrange("p (j a) -> p j a", a=8)
                nc.vector.tensor_copy(itv[:, :, 0, :], pv)
                nc.vector.tensor_copy(itv[:, :, 1, :], pvh)
                idx16.append(it)

            # ---- phase 1: per-band transpose to pixel-major DRAM scratch ----
            fmt = []
            for b in range(B):
                fm_t = dpool.tile([NPIX, C], F32, tag=f"fmt{b}")
                fmt.append(fm_t)
                for r in range(SPAN):
                    rin = rpool.tile([P, W2], F32, tag="rin")
                    nc.sync.dma_start(out=rin[:], in_=fms[:, b * SPAN * W2 + r * W2:b * SPAN * W2 + (r + 1) * W2])
                    rout = rpool.tile([P, W2], F32, tag="rout")
                    for g, gw in enumerate((4, 4, 2)):
                        pt = ppool1.tile([P, gw * P], F32, tag="pt1")
                        for q in range(gw):
                            i = g * 4 + q
                            nc.tensor.transpose(
                                pt[:, q * P:(q + 1) * P],
                                rin[:, i * P:(i + 1) * P],
                                ident[:],
                            )
                        nc.scalar.copy(rout[:, g * 512:g * 512 + gw * P], pt[:])
                    dst = fm_t[r * W2:(r + 1) * W2, :].rearrange(
                        "(i p) c -> p i c", p=P)
                    src = rout[:].rearrange("p (i c) -> p i c", c=P)
                    nc.scalar.dma_start(out=dst, in_=src)

            # ---- phases 2+3: gather, blend, transpose back, write out ----
            nchunks = (Tb + CT - 1) // CT
            for b in range(B):
                src_ap = bass.AP(fmt[b][:].tensor, 0, [[P, NPIX - 1], [1, 256]])
                for ci in range(nchunks):
                    t0 = ci * CT
                    ct = min(CT, Tb - t0)
                    ni = ct * 256
                    G = gpool.tile([P, CT * 2 * 256], F32, tag="G")
                    nc.gpsimd.dma_gather(
                        out_ap=G[:, :ct * 512].rearrange(
                            "p (n e) -> p n e", e=256),
                        in_ap=src_ap,
                        idxs_ap=idx16[b][:, t0 * 16:t0 * 16 + ct * 16],
                        num_idxs=ni,
                        num_idxs_reg=ni,
                        elem_size=256,
                        elem_step=P,
                    )
                    Gv = G[:]
                    ob = opool.tile([P, CT * P], F32, tag="ob")
                    for j in range(ct):
                        tcol = b * Tb + t0 + j
                        g00 = Gv[:, (2 * j) * 256:(2 * j) * 256 + 128]
                        g01 = Gv[:, (2 * j) * 256 + 128:(2 * j) * 256 + 256]
                        g10 = Gv[:, (2 * j + 1) * 256:(2 * j + 1) * 256 + 128]
                        g11 = Gv[:, (2 * j + 1) * 256 + 128:(2 * j + 1) * 256 + 256]
                        a0 = apool.tile([P, P], F32, tag="acc_a")
                        a1 = apool.tile([P, P], F32, tag="acc_b")
                        a2 = apool.tile([P, P], F32, tag="acc_c")
                        a3 = apool.tile([P, P], F32, tag="acc_d")
                        nc.vector.tensor_scalar_mul(a0[:], g00, w00[:, tcol:tcol + 1])
                        nc.vector.scalar_tensor_tensor(
                            a1[:], g10, w10[:, tcol:tcol + 1], a0[:],
                            op0=mybir.AluOpType.mult, op1=mybir.AluOpType.add)
                        nc.vector.scalar_tensor_tensor(
                            a2[:], g01, w01[:, tcol:tcol + 1], a1[:],
                            op0=mybir.AluOpType.mult, op1=mybir.AluOpType.add)
                        nc.vector.scalar_tensor_tensor(
                            a3[:], g11, w11[:, tcol:tcol + 1], a2[:],
                            op0=mybir.AluOpType.mult, op1=mybir.AluOpType.add)
                        ps = ppool2.tile([P, P], F32, tag="ps2")
                        nc.tensor.transpose(ps[:], a3[:], ident[:])
                        nc.scalar.copy(ob[:, j * P:(j + 1) * P], ps[:])
                    c0 = (b * Tb + t0) * P
                    nc.sync.dma_start(out=out[:, c0:c0 + ct * P],
                                      in_=ob[:, :ct * P])

    nc.compile()
    return nc


_PROGRAM_CACHE = {}


def _get_program(B, SPAN, Tb):
    key = (B, SPAN, Tb)
    if key not in _PROGRAM_CACHE:
        _PROGRAM_CACHE[key] = build_program(B, SPAN, Tb)
    return _PROGRAM_CACHE[key]


def kernel(feature_map, target_uv, downscale):
    fm = np.asarray(feature_map, dtype=np.float32)
    uv = np.asarray(target_uv, dtype=np.float32)
    ds = np.float32(np.asarray(downscale).item() if hasattr(downscale, "item")
                    else downscale)
    Cc, H, W = fm.shape
    N = uv.shape[0]
    assert Cc == C

    u = (uv[:, 0] / ds).astype(np.float32)
    v = (uv[:, 1] / ds).astype(np.float32)

    order = np.argsort(v, kind="stable")
    # per-core contiguous chunks of the sorted order
    core_bounds = [(N * k) // N_CORES for k in range(N_CORES + 1)]
    max_core_n = max(core_bounds[k + 1] - core_bounds[k] for k in range(N_CORES))

    # choose band count B so that every band spans <= 25 map rows
    B = 2
    while True:
        Tb = int(np.ceil(np.ceil(max_core_n / B) / P))
        Tb = max(Tb, 1)
        NB = Tb * P
        # compute per-(core, band) base row and span
        bases = np.zeros((N_CORES, B), dtype=np.int64)
        spans = []
        band_pts = {}   # (k, b) -> original point ids (padded to NB)
        ok = True
        for k in range(N_CORES):
            ids = order[core_bounds[k]:core_bounds[k + 1]]
            nb_bounds = [(len(ids) * b) // B for b in range(B + 1)]
            for b in range(B):
                bids = ids[nb_bounds[b]:nb_bounds[b + 1]]
                if len(bids) == 0:
                    bids = ids[:1] if len(ids) else np.array([0], dtype=np.int64)
                vb = v[bids]
                base = int(np.floor(vb.min()))
                span = int(np.floor(vb.max())) + 2 - base
                if span > 25:
                    ok = False
                    break
                bases[k, b] = base
                spans.append(span)
                pad = NB - len(bids)
                band_pts[(k, b)] = np.concatenate(
                    [bids, np.repeat(bids[:1], pad)]) if pad else bids
            if not ok:
                break
        if ok:
            break
        B += 1

    SPAN = max(spans)
    # clamp bases so base + SPAN <= H (shift coverage window up; still valid)
    for k in range(N_CORES):
        for b in range(B):
            bases[k, b] = min(bases[k, b], H - SPAN)
    Tc = B * Tb
    NCAP = Tc * P
    JW = Tb * 8

    # padded feature map width 1248 -> 1280
    fm_pad = np.zeros((C, H, W2), dtype=np.float32)
    fm_pad[:, :, :W] = fm

    in_maps = []
    for k in range(N_CORES):
        fms_k = np.empty((C, B * SPAN * W2), dtype=np.float32)
        uvt_k = np.empty((P, 2 * Tc), dtype=np.float32)
        uvw_k = np.empty((P, B * 2 * JW), dtype=np.float32)
        for b in range(B):
            base = bases[k, b]
            fms_k[:, b * SPAN * W2:(b + 1) * SPAN * W2] = \
                fm_pad[:, base:base + SPAN, :].reshape(C, SPAN * W2)
            ids = band_pts[(k, b)]
            ub = u[ids]
            vb = (v[ids] - np.float32(base)).astype(np.float32)
            # blend layout: point q = t*128 + p -> [p, t]
            ut = ub.reshape(Tb, P).T
            vt = vb.reshape(Tb, P).T
            uvt_k[:, b * Tb:(b + 1) * Tb] = ut
            uvt_k[:, Tc + b * Tb:Tc + (b + 1) * Tb] = vt
            # wrapped layout: q = j*128 + 16a + p2 -> [p2, j*8 + a], replicated
            uw = ub.reshape(Tb, 8, 16).transpose(2, 0, 1).reshape(16, JW)
            vw = vb.reshape(Tb, 8, 16).transpose(2, 0, 1).reshape(16, JW)
            uvw_k[:, b * 2 * JW:b * 2 * JW + JW] = np.tile(uw, (8, 1))
            uvw_k[:, b * 2 * JW + JW:(b + 1) * 2 * JW] = np.tile(vw, (8, 1))
        in_maps.append({"fms": fms_k, "uv_t": uvt_k, "uv_w": uvw_k})

    nc = _get_program(B, SPAN, Tb)
    res = run_bass_kernel_spmd(nc, in_maps, list(range(N_CORES)))

    out_full = np.empty((C, N), dtype=np.float32)
    for k in range(N_CORES):
        ok_arr = res.results[k]["out"]
        for b in range(B):
            ids = band_pts[(k, b)]
            idsb = order[core_bounds[k]:core_bounds[k + 1]]
            nb_bounds = [(len(idsb) * bb) // B for bb in range(B + 1)]
            nreal = nb_bounds[b + 1] - nb_bounds[b]
            cols = ok_arr[:, b * Tb * P: b * Tb * P + nreal]
            out_full[:, ids[:nreal]] = cols
    return out_full


# revision 6
# speedup vs baseline: 32055.8320x; 32055.8320x over previous
"""Bilinear resampling kernel for Trainium2 (8 NeuronCores, SPMD).

reference semantics:
    u = target_uv[:, 0] / downscale ; v = target_uv[:, 1] / downscale
    out[c, i] = bilinear sample of feature_map[c] at (v[i], u[i])   -> [C, N]

Strategy
--------
Host: sort points by v, split into 8 equal per-core chunks, then split each
core's chunk into B equal "bands" (each spanning <= ~24 feature-map rows so
that pixel indices within a band fit in int16 for dma_gather).  Each band's
map rows are sliced out (width padded 1248->1280) and shipped per core.

Device (same program on all 8 cores):
  phase 1: transpose the band map slices [C=128, SPAN*1280] -> [SPAN*1280, C]
           (pixel-major, 512B per pixel) into DRAM scratch via PE transposes.
  phase 2: per chunk of points, one dma_gather pulls a 1KB window per
           (point, v-row): the two u-corner pixel vectors, contiguous.
  phase 3: blend with per-partition scalar weights (4 fused DVE ops/tile),
           PE-transpose each [pts, C] tile back to [C, pts], copy via ScalarE
           into an output buffer, DMA to out.

Layouts (per band, Tb tiles of 128 points):
  gather i = j*256 + c*128 + p   (j tile, c in {v_lo, v_hi}, p partition)
  dma_gather output[p, i//128, :] = window  ->  G[p, 2j+c, 0:256]
  idx int16 at [i%16, i//16]  ->  col 16j + 8c + a  for p = 16a + p2;
  host pre-wraps uv accordingly (replicated over the 8 partition groups so
  the computed idx tile is replicated as the HW requires).
"""

import numpy as np

import concourse.bacc as bacc
import concourse.bass as bass
import concourse.mybir as mybir
import concourse.tile as tile
from concourse.bass_utils import run_bass_kernel_spmd
from concourse.masks import make_identity

C = 128
P = 128
W2 = 1280          # padded row pitch in pixels
N_CORES = 8
CT = 4             # tiles (of 128 points) per gather chunk -> 1024 idxs (SWDGE ring cap)
F32 = mybir.dt.float32
I16 = mybir.dt.int16
I32 = mybir.dt.int32


def _floor_ops(nc, pool, x_ap, n, lo_out, frac_out=None):
    """lo_out = floor(x), frac_out = x - floor(x). x >= 0. Exact regardless of
    the HW float->int rounding mode."""
    xi = pool.tile([P, n], I32, tag="flt_xi")
    xf = pool.tile([P, n], F32, tag="flt_xf")
    gt = pool.tile([P, n], F32, tag="flt_gt")
    nc.vector.tensor_copy(xi[:], x_ap)                   # f32 -> i32 (round?)
    nc.vector.tensor_copy(xf[:], xi[:])                  # i32 -> f32 exact
    nc.vector.tensor_tensor(gt[:], xf[:], x_ap, op=mybir.AluOpType.is_gt)
    nc.vector.tensor_tensor(lo_out, xf[:], gt[:], op=mybir.AluOpType.subtract)
    if frac_out is not None:
        nc.vector.tensor_tensor(frac_out, x_ap, lo_out, op=mybir.AluOpType.subtract)


def build_program(B, SPAN, Tb, num_devices=N_CORES, do_phase1=True, do_phase23=True,
                  rbufs=4, gbufs=3, obufs=3, abufs=12, p2bufs=5):
    """Build the SPMD Bass program. Returns (nc, io names)."""
    Tc = B * Tb                 # total tiles per core
    NCAP = Tc * P               # point capacity per core
    JW = Tb * 8                 # wrapped (j, a) cols per band
    IDXW = Tb * 16              # idx cols per band
    NPIX = SPAN * W2            # pixels per band table

    nc = bacc.Bacc("TRN2", target_bir_lowering=False, debug=False,
                   num_devices=num_devices)

    fms = nc.dram_tensor("fms", [C, B * SPAN * W2], F32, kind="ExternalInput")
    uv_t = nc.dram_tensor("uv_t", [P, 2 * Tc], F32, kind="ExternalInput")
    uv_w = nc.dram_tensor("uv_w", [P, B * 2 * JW], F32, kind="ExternalInput")
    out = nc.dram_tensor("out", [C, NCAP], F32, kind="ExternalOutput")

    with tile.TileContext(nc) as tc:
        with (
            tc.tile_pool(name="const", bufs=1) as cpool,
            tc.tile_pool(name="wbuf", bufs=1) as wpool,
            tc.tile_pool(name="scratch", bufs=2) as spool,
            tc.tile_pool(name="rowio", bufs=rbufs) as rpool,
            tc.tile_pool(name="gather", bufs=gbufs) as gpool,
            tc.tile_pool(name="acc", bufs=abufs) as apool,
            tc.tile_pool(name="obuf", bufs=obufs) as opool,
            tc.tile_pool(name="psum1", bufs=3, space="PSUM") as ppool1,
            tc.tile_pool(name="psum2", bufs=p2bufs, space="PSUM") as ppool2,
            tc.tile_pool(name="dram", bufs=1, space="DRAM") as dpool,
        ):
            ident = cpool.tile([P, P], F32, tag="ident")
            make_identity(nc, ident[:])

            # ---- weights preamble: w00..w11 [P, Tc] ----
            uvt = wpool.tile([P, 2 * Tc], F32, tag="uvt")
            nc.sync.dma_start(out=uvt[:], in_=uv_t[:])
            du = wpool.tile([P, Tc], F32, tag="du")
            dv = wpool.tile([P, Tc], F32, tag="dv")
            om_du = wpool.tile([P, Tc], F32, tag="om_du")
            om_dv = wpool.tile([P, Tc], F32, tag="om_dv")
            lo_tmp = wpool.tile([P, Tc], F32, tag="lo_tmp")
            _floor_ops(nc, spool, uvt[:, :Tc], Tc, lo_tmp[:], du[:])
            _floor_ops(nc, spool, uvt[:, Tc:], Tc, lo_tmp[:], dv[:])
            nc.vector.tensor_scalar(om_du[:], du[:], -1.0, 1.0,
                                    op0=mybir.AluOpType.mult,
                                    op1=mybir.AluOpType.add)
            nc.vector.tensor_scalar(om_dv[:], dv[:], -1.0, 1.0,
                                    op0=mybir.AluOpType.mult,
                                    op1=mybir.AluOpType.add)
            w00 = wpool.tile([P, Tc], F32, tag="w00")
            w10 = wpool.tile([P, Tc], F32, tag="w10")
            w01 = wpool.tile([P, Tc], F32, tag="w01")
            w11 = wpool.tile([P, Tc], F32, tag="w11")
            nc.vector.tensor_tensor(w00[:], om_dv[:], om_du[:], op=mybir.AluOpType.mult)
            nc.vector.tensor_tensor(w10[:], dv[:], om_du[:], op=mybir.AluOpType.mult)
            nc.vector.tensor_tensor(w01[:], om_dv[:], du[:], op=mybir.AluOpType.mult)
            nc.vector.tensor_tensor(w11[:], dv[:], du[:], op=mybir.AluOpType.mult)

            # ---- per-band gather indices (int16, wrapped+replicated) ----
            idx16 = []
            for b in range(B):
                uvw = spool.tile([P, 2 * JW], F32, tag="uvw")
                nc.sync.dma_start(out=uvw[:], in_=uv_w[:, b * 2 * JW:(b + 1) * 2 * JW])
                ulo = spool.tile([P, JW], F32, tag="ulo")
                vlo = spool.tile([P, JW], F32, tag="vlo")
                _floor_ops(nc, spool, uvw[:, :JW], JW, ulo[:])
                _floor_ops(nc, spool, uvw[:, JW:], JW, vlo[:])
                pix = spool.tile([P, JW], F32, tag="pix")
                nc.vector.tensor_scalar_mul(pix[:], vlo[:], float(W2))
                nc.vector.tensor_tensor(pix[:], pix[:], ulo[:], op=mybir.AluOpType.add)
                pix_hi = spool.tile([P, JW], F32, tag="pix_hi")
                nc.vector.tensor_scalar_add(pix_hi[:], pix[:], float(W2))
                it = wpool.tile([P, IDXW], I16, tag=f"idx{b}")
                itv = it[:].rearrange("p (j c a) -> p j c a", c=2, a=8)
                pv = pix[:].rearrange("p (j a) -> p j a", a=8)
                pvh = pix_hi[:].rearrange("p (j a) -> p j a", a=8)
                nc.vector.tensor_copy(itv[:, :, 0, :], pv)
                nc.vector.tensor_copy(itv[:, :, 1, :], pvh)
                idx16.append(it)

            # ---- phase 1: per-band transpose to pixel-major DRAM scratch ----
            fmt = []
            for b in range(B):
                fm_t = dpool.tile([NPIX, C], F32, tag=f"fmt{b}")
                fmt.append(fm_t)
                for r in range(SPAN if do_phase1 else 0):
                    rin = rpool.tile([P, W2], F32, tag="rin")
                    nc.sync.dma_start(out=rin[:], in_=fms[:, b * SPAN * W2 + r * W2:b * SPAN * W2 + (r + 1) * W2])
                    rout = rpool.tile([P, W2], F32, tag="rout")
                    for g, gw in enumerate((4, 4, 2)):
                        pt = ppool1.tile([P, gw * P], F32, tag="pt1")
                        for q in range(gw):
                            i = g * 4 + q
                            nc.tensor.transpose(
                                pt[:, q * P:(q + 1) * P],
                                rin[:, i * P:(i + 1) * P],
                                ident[:],
                            )
                        nc.scalar.copy(rout[:, g * 512:g * 512 + gw * P], pt[:])
                    dst = fm_t[r * W2:(r + 1) * W2, :].rearrange(
                        "(i p) c -> p i c", p=P)
                    src = rout[:].rearrange("p (i c) -> p i c", c=P)
                    nc.scalar.dma_start(out=dst, in_=src)

            # ---- phases 2+3: gather, blend, transpose back, write out ----
            nchunks = (Tb + CT - 1) // CT
            for b in range(B if do_phase23 else 0):
                src_ap = bass.AP(fmt[b][:].tensor, 0, [[P, NPIX - 1], [1, 256]])
                for ci in range(nchunks):
                    t0 = ci * CT
                    ct = min(CT, Tb - t0)
                    ni = ct * 256
                    G = gpool.tile([P, CT * 2 * 256], F32, tag="G")
                    nc.gpsimd.dma_gather(
                        out_ap=G[:, :ct * 512].rearrange(
                            "p (n e) -> p n e", e=256),
                        in_ap=src_ap,
                        idxs_ap=idx16[b][:, t0 * 16:t0 * 16 + ct * 16],
                        num_idxs=ni,
                        num_idxs_reg=ni,
                        elem_size=256,
                        elem_step=P,
                    )
                    Gv = G[:]
                    ob = opool.tile([P, CT * P], F32, tag="ob")
                    for j in range(ct):
                        tcol = b * Tb + t0 + j
                        g00 = Gv[:, (2 * j) * 256:(2 * j) * 256 + 128]
                        g01 = Gv[:, (2 * j) * 256 + 128:(2 * j) * 256 + 256]
                        g10 = Gv[:, (2 * j + 1) * 256:(2 * j + 1) * 256 + 128]
                        g11 = Gv[:, (2 * j + 1) * 256 + 128:(2 * j + 1) * 256 + 256]
                        a0 = apool.tile([P, P], F32, tag="acc_a")
                        a1 = apool.tile([P, P], F32, tag="acc_b")
                        a2 = apool.tile([P, P], F32, tag="acc_c")
                        a3 = apool.tile([P, P], F32, tag="acc_d")
                        nc.vector.tensor_scalar_mul(a0[:], g00, w00[:, tcol:tcol + 1])
                        nc.vector.scalar_tensor_tensor(
                            a1[:], g10, w10[:, tcol:tcol + 1], a0[:],
                            op0=mybir.AluOpType.mult, op1=mybir.AluOpType.add)
                        nc.vector.scalar_tensor_tensor(
                            a2[:], g01, w01[:, tcol:tcol + 1], a1[:],
                            op0=mybir.AluOpType.mult, op1=mybir.AluOpType.add)
                        nc.vector.scalar_tensor_tensor(
                            a3[:], g11, w11[:, tcol:tcol + 1], a2[:],
                            op0=mybir.AluOpType.mult, op1=mybir.AluOpType.add)
                        ps = ppool2.tile([P, P], F32, tag="ps2")
                        nc.tensor.transpose(ps[:], a3[:], ident[:])
                        nc.scalar.copy(ob[:, j * P:(j + 1) * P], ps[:])
                    c0 = (b * Tb + t0) * P
                    nc.sync.dma_start(out=out[:, c0:c0 + ct * P],
                                      in_=ob[:, :ct * P])

    nc.compile()
    return nc


_PROGRAM_CACHE = {}


def _get_program(B, SPAN, Tb):
    key = (B, SPAN, Tb)
    if key not in _PROGRAM_CACHE:
        _PROGRAM_CACHE[key] = build_program(B, SPAN, Tb)
    return _PROGRAM_CACHE[key]


def kernel(feature_map, target_uv, downscale):
    fm = np.asarray(feature_map, dtype=np.float32)
    uv = np.asarray(target_uv, dtype=np.float32)
    ds = np.float32(np.asarray(downscale).item() if hasattr(downscale, "item")
                    else downscale)
    Cc, H, W = fm.shape
    N = uv.shape[0]
    assert Cc == C

    u = (uv[:, 0] / ds).astype(np.float32)
    v = (uv[:, 1] / ds).astype(np.float32)

    order = np.argsort(v, kind="stable")
    # per-core contiguous chunks of the sorted order
    core_bounds = [(N * k) // N_CORES for k in range(N_CORES + 1)]
    max_core_n = max(core_bounds[k + 1] - core_bounds[k] for k in range(N_CORES))

    # choose band count B so that every band spans <= 25 map rows
    B = 2
    while True:
        Tb = int(np.ceil(np.ceil(max_core_n / B) / P))
        Tb = max(Tb, 1)
        NB = Tb * P
        # compute per-(core, band) base row and span
        bases = np.zeros((N_CORES, B), dtype=np.int64)
        spans = []
        band_pts = {}   # (k, b) -> original point ids (padded to NB)
        ok = True
        for k in range(N_CORES):
            ids = order[core_bounds[k]:core_bounds[k + 1]]
            nb_bounds = [(len(ids) * b) // B for b in range(B + 1)]
            for b in range(B):
                bids = ids[nb_bounds[b]:nb_bounds[b + 1]]
                if len(bids) == 0:
                    bids = ids[:1] if len(ids) else np.array([0], dtype=np.int64)
                vb = v[bids]
                base = int(np.floor(vb.min()))
                span = int(np.floor(vb.max())) + 2 - base
                if span > 25:
                    ok = False
                    break
                bases[k, b] = base
                spans.append(span)
                pad = NB - len(bids)
                band_pts[(k, b)] = np.concatenate(
                    [bids, np.repeat(bids[:1], pad)]) if pad else bids
            if not ok:
                break
        if ok:
            break
        B += 1

    SPAN = max(spans)
    # clamp bases so base + SPAN <= H (shift coverage window up; still valid)
    for k in range(N_CORES):
        for b in range(B):
            bases[k, b] = min(bases[k, b], H - SPAN)
    Tc = B * Tb
    NCAP = Tc * P
    JW = Tb * 8

    # padded feature map width 1248 -> 1280
    fm_pad = np.zeros((C, H, W2), dtype=np.float32)
    fm_pad[:, :, :W] = fm

    in_maps = []
    for k in range(N_CORES):
        fms_k = np.empty((C, B * SPAN * W2), dtype=np.float32)
        uvt_k = np.empty((P, 2 * Tc), dtype=np.float32)
        uvw_k = np.empty((P, B * 2 * JW), dtype=np.float32)
        for b in range(B):
            base = bases[k, b]
            fms_k[:, b * SPAN * W2:(b + 1) * SPAN * W2] = \
                fm_pad[:, base:base + SPAN, :].reshape(C, SPAN * W2)
            ids = band_pts[(k, b)]
            ub = u[ids]
            vb = (v[ids] - np.float32(base)).astype(np.float32)
            # blend layout: point q = t*128 + p -> [p, t]
            ut = ub.reshape(Tb, P).T
            vt = vb.reshape(Tb, P).T
            uvt_k[:, b * Tb:(b + 1) * Tb] = ut
            uvt_k[:, Tc + b * Tb:Tc + (b + 1) * Tb] = vt
            # wrapped layout: q = j*128 + 16a + p2 -> [p2, j*8 + a], replicated
            uw = ub.reshape(Tb, 8, 16).transpose(2, 0, 1).reshape(16, JW)
            vw = vb.reshape(Tb, 8, 16).transpose(2, 0, 1).reshape(16, JW)
            uvw_k[:, b * 2 * JW:b * 2 * JW + JW] = np.tile(uw, (8, 1))
            uvw_k[:, b * 2 * JW + JW:(b + 1) * 2 * JW] = np.tile(vw, (8, 1))
        in_maps.append({"fms": fms_k, "uv_t": uvt_k, "uv_w": uvw_k})

    nc = _get_program(B, SPAN, Tb)
    res = run_bass_kernel_spmd(nc, in_maps, list(range(N_CORES)))

    out_full = np.empty((C, N), dtype=np.float32)
    for k in range(N_CORES):
        ok_arr = res.results[k]["out"]
        for b in range(B):
            ids = band_pts[(k, b)]
            idsb = order[core_bounds[k]:core_bounds[k + 1]]
            nb_bounds = [(len(idsb) * bb) // B for bb in range(B + 1)]
            nreal = nb_bounds[b + 1] - nb_bounds[b]
            cols = ok_arr[:, b * Tb * P: b * Tb * P + nreal]
            out_full[:, ids[:nreal]] = cols
    return out_full
